# revision 34
# baseline (speedup 1.0000x reference)
"""Trainium2 Bass kernel for nn_DiffusionActionHead (MoE-style category routing).

Strategy (host side, inside kernel()):
  - Group the B=32 batch items by cat_id into token groups of <= IPS items.
    Each group's work is split into two column-halves (output-dim split of the
    big matmuls), giving uniform "half-unit" slots. Slots are distributed
    round-robin over the 8 NeuronCores; every core runs the SAME program over
    NSLOT slots (SPMD). Dummy padding slots replicate slot 0; outputs discarded.
  - Weight tables are cast to bf16, and the three lowest-sensitivity tables
    (se_W2, ae_W2) to E3M4 fp8, with power-of-2 scales folded into neighbouring
    tensors host-side so the device needs no descale ops beyond the sigmoid's
    ACT scale. This quarters HBM traffic vs fp32 — the kernel is
    DMA-bandwidth-bound (~310-360 GB/s/core effective).
  - Everything one slot needs from one HWDGE queue is packed into a single
    byte-blob (uint8) DMA'd in one shot: 3 DMAs per slot total (SP blob, ACT
    blob, output) instead of 12, hiding per-DMA queue overhead (~650ns each).
  - Per-item sinusoidal timestep embeddings are computed on host (function of
    the int timesteps input only); all weight-table FLOPs run on device.
  - Column-half partial outputs are summed on host during unsharding.

Device program per slot (raw Bass, manual semaphores):
  SE1  hT = relu(seW1h^T @ state + b1h)        (4x [128,IPS] matmuls)
  SE2  sf = hT^T @ seW2h (+ sb2)               (partial state_feat, 3 o-tiles)
  AE1  aT = (W1 chunks)^T @ actionsT + b1      (12x [128,TOK], transposed out)
  TT   tt = tauT^T @ W2bh (+ b2)               (per-item tau contribution)
  X2T  x2T = W2ah^T @ aT + broadcast(tt)^T; swish   (6 o-chunks of [128,TOK],
       weights stationary - output lands pre-transposed, no TR phase)
  AE3  out = x2T^T @ W3h (+ b3)                (partial, 3 o-tiles of 512)
"""
import sys

sys.path.insert(0, "/opt/trn_rl_repo")

import contextlib
import numpy as np
import ml_dtypes

import concourse.bass as bass
import concourse.mybir as mybir
from concourse.bass_utils import run_bass_kernel_spmd

F32 = mybir.dt.float32
F32R = mybir.dt.float32r
BF16 = mybir.dt.bfloat16
FP8 = mybir.dt.float8e3
U8 = mybir.dt.uint8
AF = mybir.ActivationFunctionType

NP_BF16 = ml_dtypes.bfloat16
NP_FP8 = ml_dtypes.float8_e3m4

E, STATE_DIM, ACT_DIM, HID, EMB = 32, 64, 32, 1024, 1536
B, T = 32, 32
N_CORES = 8
HH = HID // 2               # 512: h-column half for the state encoder
OH = EMB // 2               # 768: output-column half for the action encoder
RS = 3                      # SP-queue blob ring depth
RA = 3                      # ACT-queue blob ring depth

# ---- dtype config -----------------------------------------------------------
# E3M4 fp8 (4 mantissa bits, ~1.3% RMS) for the low-sensitivity tables; bf16
# (~0.17% RMS) elsewhere. Measured end-to-end rel err 1.40e-2 vs the 2e-2 gate.
FP8_WSE2 = True
FP8_W2A = True
FP8_W2B = True
MIXED = True        # bf16 stationary operand against fp8 moving operand (and
                    # vice versa) — validated on HW in both directions
S_W = 128           # fp8 quantization scale (sigma 0.02 -> 2.56)

S_AT = 32 if (FP8_W2A and not MIXED) else 1      # aT fp8 needs its own scale
S_H = 16 if (FP8_WSE2 and not MIXED) else 1      # h fp8 scale
S2 = (S_W if FP8_W2A else 1) * S_AT              # scale carried by X2 psum
S1 = (S_W if FP8_WSE2 else 1) * S_H              # scale carried by SE2 psum
TAU_SCALE = S2 / (S_W if FP8_W2B else 1)         # host scales tau by this
assert not (FP8_W2B and not MIXED), "fp8 W2b needs bf16 tau (mixed matmul)"

SZ_SE2 = 1 if FP8_WSE2 else 2
SZ_2B = 1 if FP8_W2B else 2
SZ_2A = 1 if FP8_W2A else 2

AT_DT = FP8 if (FP8_W2A and not MIXED) else BF16
HT_DT = FP8 if (FP8_WSE2 and not MIXED) else BF16


def _align16(x):
    return (x + 15) & ~15


def _blob_offsets(ips):
    """Byte offsets of each table inside the SP and ACT per-slot blobs."""
    TOK = ips * T
    pin_w = _align16(48 + TOK + ips)            # elems (bf16)
    o = {}
    o["PIN_W"] = pin_w
    o["P"] = 0                                   # pin, bf16 [128, pin_w]
    o["W"] = _align16(pin_w * 2)                 # wsea, bf16 [96, 1024]
    o["S2"] = _align16(o["W"] + 2048)            # wse2 g0 [128, 3072]
    o["2B"] = _align16(o["S2"] + 3072 * SZ_SE2)  # w2b g0 [128, 4608]
    o["2A"] = _align16(o["2B"] + 4608 * SZ_2B)   # w2a g0 [128, 4608]
    o["3"] = _align16(o["2A"] + 4608 * SZ_2A)    # w3 g0, bf16 [128, 4608]
    o["SPW"] = _align16(o["3"] + 9216)
    o["aS2"] = 0
    o["a2B"] = _align16(3072 * SZ_SE2)
    o["a2A"] = _align16(o["a2B"] + 4608 * SZ_2B)
    o["a3"] = _align16(o["a2A"] + 4608 * SZ_2A)
    o["ACW"] = _align16(o["a3"] + 9216)
    return o


def _sinusoid(ts):
    half = EMB // 2
    div = np.exp(-np.log(np.float32(10000.0)) * np.arange(half, dtype=np.float32) / np.float32(half))
    ang = ts.astype(np.float32)[:, None] * div[None, :]
    return np.concatenate([np.sin(ang), np.cos(ang)], axis=1).astype(np.float32)


def _bf16(a):
    return np.asarray(a, np.float32).astype(NP_BF16)


def _fp8(a, scale):
    return np.clip(np.asarray(a, np.float32) * scale, -15.0, 15.0).astype(NP_FP8)


# ---------------------------------------------------------------------------
# Build-time plan. Ops live in engine streams: "dma" (SP: blob DMAs), "pe"
# (matmuls), "actq" (ACT: activations, ACT-queue blob DMAs, output DMAs),
# "dve". Sem protocol: every DMA incs its per-buffer sem by 16; every PE op
# incs s_pe by 1; every activation incs s_act by 1; every DVE op incs s_dve
# by 1. Cross-engine deps become standalone wait_ge ops. DMA completion tags
# use the pseudo-stream "dma-async" so consumers always wait the sem even from
# the issuing engine's own stream.
# ---------------------------------------------------------------------------
class _Buf:
    __slots__ = ("writer", "readers")

    def __init__(self):
        self.writer = None      # (sem, value, stream)
        self.readers = []


class _Plan:
    def __init__(self):
        self.dma = []
        self.pe = []
        self.actq = []
        self.dve = []
        self.counts = {}

    def emit(self, stream, sem, mult, op, in_bufs, out_buf, force_wait=False,
             tag_stream=None):
        self.counts[sem] = self.counts.get(sem, 0) + 1
        tag = (sem, self.counts[sem] * mult, tag_stream or stream)
        deps = []
        for b in in_bufs:
            if b.writer is not None:
                deps.append(b.writer)
        if out_buf is not None:
            deps.extend(out_buf.readers)
            if out_buf.writer is not None:
                deps.append(out_buf.writer)
        m = {}
        for dsem, dval, dstream in deps:
            if dstream == stream and not force_wait:
                continue  # same engine stream: program order
            m[dsem] = max(m.get(dsem, 0), dval)
        op["waits"] = m
        getattr(self, stream).append(op)
        for b in in_bufs:
            b.readers.append(tag)
        if out_buf is not None:
            out_buf.writer = tag
            out_buf.readers = []


def build(nslot, reps=1, ips=2, with_bias=False, probe=None):
    TOK = ips * T
    OFF = _blob_offsets(ips)
    PIN_W = OFF["PIN_W"]
    SPW, ACW = OFF["SPW"], OFF["ACW"]
    # element offsets inside the bf16 / fp8 views of the blobs
    EPIN, EWSEA, EW3 = OFF["P"] // 2, OFF["W"] // 2, OFF["3"] // 2
    ES2 = OFF["S2"] // SZ_SE2
    E2B = OFF["2B"] // SZ_2B
    E2A = OFF["2A"] // SZ_2A
    AS2 = OFF["aS2"] // SZ_SE2
    A2B = OFF["a2B"] // SZ_2B
    A2A = OFF["a2A"] // SZ_2A
    AW3 = OFF["a3"] // 2
    PIN_TAU, PIN_ACT = 0, 48
    PIN_ST = PIN_ACT + TOK
    V_SE2 = ("r8" if FP8_WSE2 else "rb", "a8" if FP8_WSE2 else "ab")
    V_2B = ("r8" if FP8_W2B else "rb", "a8" if FP8_W2B else "ab")
    V_2A = ("r8" if FP8_W2A else "rb", "a8" if FP8_W2A else "ab")

    nc = bass.Bass()
    P = nc.declare_dram_parameter

    spblob = P("spblob", [nslot, 128, SPW], U8, isOutput=False)
    acblob = P("acblob", [nslot, 128, ACW], U8, isOutput=False)
    cst_f = P("cst_f", [128, 256], F32R, isOutput=False)           # onesel|ones
    biasd = (P("biasd", [nslot, 128, 3872], F32R, isOutput=False)
             if with_bias else None)
    merged = (TOK + ips) <= 128
    oo = P("o", [nslot, TOK + ips if merged else TOK, EMB], BF16, isOutput=True)
    stp = None if merged else P("st", [nslot, ips, EMB], BF16, isOutput=True)

    with contextlib.ExitStack() as es:
        ec = es.enter_context
        ring = [ec(nc.sbuf_tensor(f"ring{i}", [128, SPW], U8)) for i in range(RS)]
        ringa = [ec(nc.sbuf_tensor(f"ringa{i}", [128, ACW], U8)) for i in range(RA)]
        bias_b = ([ec(nc.sbuf_tensor(f"bias{i}", [128, 3872], F32R)) for i in range(2)]
                  if with_bias else [])
        cstF = ec(nc.sbuf_tensor("cstF", [128, 256], F32R))
        s_hT = ec(nc.sbuf_tensor("s_hT", [128, 4 * ips], HT_DT))
        s_aT = ec(nc.sbuf_tensor("s_aT", [128, 12 * TOK], AT_DT))
        s_tt = ec(nc.sbuf_tensor("s_tt", [ips, OH], F32R))
        s_sg = ec(nc.sbuf_tensor("s_sg", [128, 6 * TOK], BF16))
        s_x2T = ec(nc.sbuf_tensor("s_x2T", [128, 6 * TOK], BF16))
        s_o = [ec(nc.sbuf_tensor(f"s_o{i}", [TOK + ips if merged else TOK, EMB], BF16))
               for i in range(2)]
        s_st = ([] if merged else
                [ec(nc.sbuf_tensor(f"s_st{i}", [ips, EMB], BF16)) for i in range(2)])
        pA = ec(nc.psum_tensor("pA", [128, 512], F32))
        pA2 = ec(nc.psum_tensor("pA2", [128, 512], F32))
        pB0 = ec(nc.psum_tensor("pB0", [128, 512], F32))
        pB1 = ec(nc.psum_tensor("pB1", [128, 512], F32))
        pC = ec(nc.psum_tensor("pC", [128, 512], F32))
        pD = ec(nc.psum_tensor("pD", [128, 512], F32))
        pE = ec(nc.psum_tensor("pE", [128, 512], F32))
        s_pe = ec(nc.semaphore("s_pe"))
        s_act = ec(nc.semaphore("s_act"))
        s_dve = ec(nc.semaphore("s_dve"))
        block = ec(nc.Block())

        rb = [r.bitcast(BF16) for r in ring]
        r8 = [r.bitcast(FP8) for r in ring]
        ab = [r.bitcast(BF16) for r in ringa]
        a8 = [r.bitcast(FP8) for r in ringa]

        # ---------------- plan ----------------
        pl = _Plan()
        bufs = {
            "ring": [_Buf() for _ in range(RS)],
            "ringa": [_Buf() for _ in range(RA)],
            "bias": [_Buf() for _ in range(2)],
            "hT": [_Buf() for _ in range(4)],
            "aT": [_Buf() for _ in range(12)],
            "tt": [_Buf() for _ in range(2)],
            "sg": [_Buf() for _ in range(6)],
            "x2T": [_Buf() for _ in range(6)],
            "o": [_Buf() for _ in range(2)],
            "stb": [_Buf() for _ in range(2)],
            # each psum tensor is a single PSUM bank: PE writes and ACT/DVE
            # reads of the same bank are fatal if concurrent (P10), so track
            # whole-tensor — each new PE write waits for the previous reader.
            "pA": _Buf(),
            "pA2": _Buf(),
            "pB0": _Buf(),
            "pB1": _Buf(),
            "pC": _Buf(),
            "pD": _Buf(),
            "pE": _Buf(),
            "cstF": _Buf(),
        }
        rc = [0]
        rca = [0]

        def next_ring():
            r = rc[0] % RS
            rc[0] += 1
            return r

        def next_ringa():
            r = rca[0] % RA
            rca[0] += 1
            return r

        def dma_in(dst, dst_sl, src, src_sl, buf, key, q="sp"):
            # per-buffer DMA sems: successive writes to one buffer are ordered
            # by the WAR chain, so "sem >= 16*n" fires exactly at write n's
            # completion; a shared cumulative sem would be unsound.
            if q == "sp":
                pl.emit("dma", "dma:" + key, 16,
                        {"dst": dst, "dst_sl": dst_sl, "src": src, "src_sl": src_sl,
                         "key": "dma:" + key},
                        [], buf, tag_stream="dma-async")
            else:
                pl.emit("actq", "dmo:" + key, 16,
                        {"kind": "dmo", "dst": dst, "dst_sl": dst_sl, "src": src,
                         "src_sl": src_sl, "key": "dmo:" + key},
                        [], buf, tag_stream="dma-async")

        def dma_out(dst, dst_sl, src, src_sl, buf, key):
            # on the ACT stream; force same-stream wait (DMA engines are async
            # w.r.t. the ACT pipeline, so wait for the producing copy's sem)
            pl.emit("actq", "dmo:" + key, 16,
                    {"kind": "dmo", "dst": dst, "dst_sl": dst_sl, "src": src,
                     "src_sl": src_sl, "key": "dmo:" + key}, [buf], None,
                    force_wait=True, tag_stream="dma-async")

        def mm(out, out_sl, lhs, lhs_sl, rhs, rhs_sl, start, stop, in_bufs, out_buf):
            pl.emit("pe", "pe", 1,
                    {"kind": "mm", "out": out, "out_sl": out_sl, "lhs": lhs,
                     "lhs_sl": lhs_sl, "rhs": rhs, "rhs_sl": rhs_sl,
                     "start": start, "stop": stop}, in_bufs, out_buf)

        def act(out, out_sl, in_, in_sl, func, bias, scale, in_bufs, out_buf):
            pl.emit("actq", "act", 1,
                    {"kind": "act", "out": out, "out_sl": out_sl, "in": in_,
                     "in_sl": in_sl, "func": func, "bias": bias, "scale": scale},
                    in_bufs, out_buf)

        def dve(out, out_sl, in_, in_sl, in_bufs, out_buf):
            pl.emit("dve", "dve", 1,
                    {"out": out, "out_sl": out_sl, "in": in_, "in_sl": in_sl},
                    in_bufs, out_buf)

        dma_in("cstF", np.s_[:, :], "cst_f", np.s_[:, :], bufs["cstF"], "cstf")
        CS_ONE = 128  # cstF col of the all-ones row (bias broadcast matmuls)

        def emit_dmas(s, rsp, rac, sb):
            # issued one slot AHEAD of the consuming compute, so every blob
            # has a full slot of time to land and the PE never waits mid-slot
            dma_in("au", (rac, np.s_[:, 0:ACW]), "acblob", np.s_[s, :, :],
                   bufs["ringa"][rac], f"a{rac}", q="act")
            dma_in("ru", (rsp, np.s_[:, 0:SPW]), "spblob", np.s_[s, :, :],
                   bufs["ring"][rsp], f"r{rsp}")
            if with_bias:
                dma_in("bias_b", (sb, np.s_[:, :]), "biasd", np.s_[s, :, :],
                       bufs["bias"][sb], f"bias{sb}")

        def emit_slot(s, rsp, rac, emit_prev_out):
            sb = s % 2
            biab = bufs["bias"][sb]
            spb = bufs["ring"][rsp]
            acb = bufs["ringa"][rac]

            # ---- SE1: hT[128h, ips] per k-chunk of the h-half ----
            # alternate two PSUM banks so the PE never waits on the previous
            # chunk's ACT read (same-bank concurrent access is fatal).
            for k in range(4):
                pn = "pA" if k % 2 == 0 else "pA2"
                mm(pn, np.s_[0:128, k * ips:(k + 1) * ips],
                   "rb", (rsp, np.s_[0:STATE_DIM, EWSEA + k * 128:EWSEA + (k + 1) * 128]),
                   "rb", (rsp, np.s_[0:STATE_DIM, EPIN + PIN_ST:EPIN + PIN_ST + ips]),
                   True, True, [spb], bufs[pn])
                act("s_hT", np.s_[:, k * ips:(k + 1) * ips],
                    pn, np.s_[0:128, k * ips:(k + 1) * ips],
                    AF.Relu, ((sb, 12 + k) if with_bias else None), float(S_H),
                    [bufs[pn]] + ([biab] if with_bias else []), bufs["hT"][k])

            # ---- SE2 + AE1 + TT, interleaved round-robin so the PE always
            # has independent matmuls between AE1's bank-limited (mm -> copy
            # -> mm) chains. Three disjoint PSUM groups run concurrently:
            # SE2 in pB0/pB1/pE, AE1 in pA/pA2, TT in pC/pD.
            nq = 512 // TOK
            plain = (S_AT == 1 and not with_bias)

            def ae1_mm(j):
                q = (j // 2) % nq
                pn = "pA" if j % 2 == 0 else "pA2"
                # ae_W1 chunk j sits at base partition 32*(j//4) — matmul
                # operand bases are restricted to {0, 32, 64}
                r0 = 32 * (j // 4)
                c0 = EWSEA + 512 + (j % 4) * 128
                mm(pn, np.s_[:, q * TOK:(q + 1) * TOK],
                   "rb", (rsp, np.s_[r0:r0 + ACT_DIM, c0:c0 + 128]),
                   "rb", (rsp, np.s_[r0:r0 + ACT_DIM, EPIN + PIN_ACT:EPIN + PIN_ACT + TOK]),
                   True, True, [spb], bufs[pn])
                if plain and j % 4 >= 2:
                    dve("s_aT", np.s_[:, j * TOK:(j + 1) * TOK],
                        pn, np.s_[:, q * TOK:(q + 1) * TOK],
                        [bufs[pn]], bufs["aT"][j])
                else:
                    act("s_aT", np.s_[:, j * TOK:(j + 1) * TOK],
                        pn, np.s_[:, q * TOK:(q + 1) * TOK],
                        AF.Copy if plain else AF.Identity,
                        ((sb, j) if with_bias else None), float(S_AT),
                        [bufs[pn]] + ([biab] if with_bias else []), bufs["aT"][j])

            for r in range(12):
                ae1_mm(r)
                if r < 4:
                    k = r
                    gi, c = divmod(k, 2)
                    if gi == 0:
                        rn, rg, off, rbuf = V_SE2[0], rsp, ES2, spb
                    else:
                        rn, rg, off, rbuf = V_SE2[1], rac, AS2, acb
                    for t, pn in enumerate(("pB0", "pB1", "pE")):
                        mm(pn, np.s_[0:ips, 0:512],
                           "s_hT", np.s_[:, k * ips:(k + 1) * ips],
                           rn, (rg, np.s_[:, off + c * 1536 + t * 512:off + c * 1536 + (t + 1) * 512]),
                           k == 0, (k == 3 and not with_bias),
                           [bufs["hT"][k], rbuf], bufs[pn])
                k = r
                gi, c = divmod(k, 6)
                if gi == 0:
                    rn, rg, off, rbuf = V_2B[0], rsp, E2B, spb
                else:
                    rn, rg, off, rbuf = V_2B[1], rac, A2B, acb
                for t, pn in enumerate(("pC", "pD")):
                    mm(pn, np.s_[0:ips, 0:384],
                       "rb", (rsp, np.s_[0:128, EPIN + PIN_TAU + k * ips:EPIN + PIN_TAU + (k + 1) * ips]),
                       rn, (rg, np.s_[:, off + c * 768 + t * 384:off + c * 768 + (t + 1) * 384]),
                       k == 0, (k == 11 and not with_bias),
                       [spb, rbuf], bufs[pn])
                if r == 3:
                    if with_bias:
                        for t, pn in enumerate(("pB0", "pB1", "pE")):
                            mm(pn, np.s_[0:ips, 0:512],
                               "cstF", np.s_[0:1, CS_ONE:CS_ONE + ips],
                               "bias_b", (sb, np.s_[0:1, 2336 + t * 512:2336 + (t + 1) * 512]),
                               False, True, [bufs["cstF"], biab], bufs[pn])
                    for t, pn in enumerate(("pB0", "pB1", "pE")):
                        if merged:
                            dve("s_o", (sb, np.s_[TOK:TOK + ips, t * 512:(t + 1) * 512]),
                                pn, np.s_[0:ips, 0:512], [bufs[pn]], bufs["o"][sb])
                        else:
                            dve("s_st", (sb, np.s_[0:ips, t * 512:(t + 1) * 512]),
                                pn, np.s_[0:ips, 0:512], [bufs[pn]], bufs["stb"][sb])
            if with_bias:
                for t, pn in enumerate(("pC", "pD")):
                    mm(pn, np.s_[0:ips, 0:384],
                       "cstF", np.s_[0:1, CS_ONE:CS_ONE + ips],
                       "bias_b", (sb, np.s_[0:1, 16 + t * 384:16 + (t + 1) * 384]),
                       False, True, [bufs["cstF"], biab], bufs[pn])
            for t, pn in enumerate(("pC", "pD")):
                act("s_tt", np.s_[0:ips, t * 384:(t + 1) * 384],
                    pn, np.s_[0:ips, 0:384], AF.Copy, None, 1.0,
                    [bufs[pn]], bufs["tt"][t])

            # previous slot's output DMA, ~2/3 of a slot into this slot
            emit_prev_out()

            # ---- X2T: per 128-wide o-chunk, W2a chunks stationary and aT
            # moving, so x2 lands already transposed ([o, tok]) and the TR
            # phase disappears. Banks alternate by o-chunk parity; the tau
            # contribution broadcasts in via an s_tt-stationary matmul.
            for oc in range(6):
                pn = "pC" if oc % 2 == 0 else "pD"
                for gi in range(2):
                    if gi == 0:
                        rn, rg, off, rbuf = V_2A[0], rsp, E2A, spb
                    else:
                        rn, rg, off, rbuf = V_2A[1], rac, A2A, acb
                    for c in range(6):
                        k = gi * 6 + c
                        mm(pn, np.s_[0:128, 0:TOK],
                           rn, (rg, np.s_[:, off + c * 768 + oc * 128:off + c * 768 + (oc + 1) * 128]),
                           "s_aT", np.s_[:, k * TOK:(k + 1) * TOK],
                           k == 0, False, [rbuf, bufs["aT"][k]], bufs[pn])
                mm(pn, np.s_[0:128, 0:TOK],
                   "s_tt", np.s_[0:ips, oc * 128:(oc + 1) * 128],
                   "cstF", np.s_[0:ips, 0:TOK],
                   False, True, [bufs["tt"][oc // 3], bufs["cstF"]], bufs[pn])
                # swish = x * sigmoid(x): ACT computes sigmoid, DVE multiplies.
                # psum carries S2*x; sigmoid descales, the product keeps S2
                # (folded into W3 host-side).
                act("s_sg", np.s_[:, oc * TOK:(oc + 1) * TOK], pn, np.s_[0:128, 0:TOK],
                    AF.Sigmoid, None, 1.0 / S2, [bufs[pn]], bufs["sg"][oc])
                pl.emit("dve", "dve", 1,
                        {"kind": "mul",
                         "out": "s_x2T", "out_sl": np.s_[:, oc * TOK:(oc + 1) * TOK],
                         "in": pn, "in_sl": np.s_[0:128, 0:TOK],
                         "in2": "s_sg", "in2_sl": np.s_[:, oc * TOK:(oc + 1) * TOK]},
                        [bufs[pn], bufs["sg"][oc]], bufs["x2T"][oc])
            # ---- AE3 (2 mega-chunks of 3 k-chunks) ----
            for gi in range(2):
                if gi == 0:
                    rn, rg, off, rbuf = "rb", rsp, EW3, spb
                else:
                    rn, rg, off, rbuf = "ab", rac, AW3, acb
                for c in range(3):
                    k = gi * 3 + c
                    for t, pn in enumerate(("pC", "pD", "pE")):
                        mm(pn, np.s_[0:TOK, 0:512], "s_x2T", np.s_[:, k * TOK:(k + 1) * TOK],
                           rn, (rg, np.s_[:, off + c * 1536 + t * 512:off + c * 1536 + (t + 1) * 512]),
                           k == 0, (k == 5 and not with_bias),
                           [bufs["x2T"][k], rbuf], bufs[pn])
            if with_bias:
                for t, pn in enumerate(("pC", "pD", "pE")):
                    mm(pn, np.s_[0:TOK, 0:512],
                       "cstF", np.s_[0:1, CS_ONE:CS_ONE + TOK],
                       "bias_b", (sb, np.s_[0:1, 800 + t * 512:800 + (t + 1) * 512]),
                       False, True, [bufs["cstF"], biab], bufs[pn])
            for t, pn in enumerate(("pC", "pD", "pE")):
                dve("s_o", (sb, np.s_[0:TOK, t * 512:(t + 1) * 512]),
                    pn, np.s_[0:TOK, 0:512], [bufs[pn]], bufs["o"][sb])

        def make_out_emitter(s):
            def f():
                sb = s % 2
                dma_out("o", np.s_[s, :, :], "s_o", (sb, np.s_[:, :]),
                        bufs["o"][sb], f"o{sb}")
                if not merged:
                    dma_out("st", np.s_[s, :, :], "s_st", (sb, np.s_[:, :]),
                            bufs["stb"][sb], f"st{sb}")
            return f

        pending = lambda: None  # noqa: E731
        total = reps * nslot
        emit_dmas(0, next_ring(), next_ringa(), 0)
        ring_of = {0: (0, 0)}
        for v in range(total):
            if v + 1 < total:
                rsp, rac = next_ring(), next_ringa()
                ring_of[v + 1] = (rsp, rac)
                emit_dmas((v + 1) % nslot, rsp, rac, (v + 1) % 2)
            emit_slot(v % nslot, *ring_of[v], pending)
            pending = make_out_emitter(v % nslot)
        pending()

        # ---------------- emit ----------------
        dma_sems = {k: ec(nc.semaphore("sem_" + k.replace(":", "_")))
                    for k in pl.counts if k.startswith(("dma:", "dmo:"))}

        tensors = {
            "ru": ring, "au": ringa, "rb": rb, "r8": r8, "ab": ab, "a8": a8,
            "bias_b": bias_b, "cstF": cstF,
            "s_hT": s_hT, "s_aT": s_aT,
            "s_tt": s_tt, "s_sg": s_sg, "s_x2T": s_x2T, "s_o": s_o, "s_st": s_st,
            "pA": pA, "pA2": pA2, "pB0": pB0, "pB1": pB1, "pC": pC, "pD": pD, "pE": pE,
            "spblob": spblob, "acblob": acblob, "biasd": biasd, "cst_f": cst_f,
            "o": oo, "st": stp,
        }

        def ap(name, sl):
            t = tensors[name]
            if isinstance(t, list):
                i, s2_ = sl
                return t[i][s2_]
            return t[sl]

        sems = {"pe": s_pe, "act": s_act, "dve": s_dve}

        def make_waiter(eng_handle):
            hw = {}

            def wait(wmap):
                for sname in sorted(wmap):
                    val = wmap[sname]
                    if hw.get(sname, 0) >= val:
                        continue
                    hw[sname] = val
                    h = sems[sname] if sname in sems else dma_sems[sname]
                    eng_handle.wait_ge(h, val)

            return wait

        if probe == "pe":
            pl.dma = []
        if probe in ("dma", "pe"):
            for _lst in (pl.dma, pl.pe, pl.actq, pl.dve):
                for _op in _lst:
                    _op["waits"] = {}
        if probe == "dma":
            # self-throttle: each DMA waits for the previous write to its own
            # buffer (ring depth flow control without compute). DVE reads each
            # landed blob into s_o, which the output DMAs drain — keeps the
            # whole chain live against dead-code elimination.
            pl.pe = []
            _in_dmas = (pl.dma + [o for o in pl.actq if o.get("kind") == "dmo"
                                  and o["dst"] not in ("o", "st")])
            for _lst in (pl.dma, pl.actq):
                _kc = {}
                for _op in _lst:
                    if _lst is pl.dma or _op.get("kind") == "dmo":
                        _k = _op["key"]
                        if _kc.get(_k, 0) > 0:
                            _op["waits"] = {_k: 16 * _kc[_k]}
                        _kc[_k] = _kc.get(_k, 0) + 1
            pl.actq = [o for o in pl.actq if o.get("kind") == "dmo"]
            _reads = []
            for op in _in_dmas:
                sl = op["dst_sl"]
                if isinstance(sl, tuple) and isinstance(sl[0], int):
                    in_sl = (sl[0], np.s_[0:4, 0:4])
                else:
                    in_sl = np.s_[0:4, 0:4]
                _reads.append({"out": "s_o", "out_sl": (0, np.s_[0:4, 0:4]),
                               "in": op["dst"], "in_sl": in_sl, "waits": {}})
            pl.dve = _reads
        if probe == "pe":
            pl.actq = []
            pl.dve = []

        @block.sync
        def _(sync):
            wait = make_waiter(sync)
            cnt = {}
            for op in pl.dma:
                wait(op["waits"])
                k = op["key"]
                cnt[k] = cnt.get(k, 0) + 16
                sync.dma_start(out=ap(op["dst"], op["dst_sl"]),
                               in_=ap(op["src"], op["src_sl"])).then_inc(dma_sems[k], 16)
            for k, v in sorted(cnt.items()):
                sync.wait_ge(dma_sems[k], v)

        @block.tensor
        def _(pe):
            wait = make_waiter(pe)
            for op in pl.pe:
                wait(op["waits"])
                pe.matmul(ap(op["out"], op["out_sl"]), ap(op["lhs"], op["lhs_sl"]),
                          ap(op["rhs"], op["rhs_sl"]), start=op["start"],
                          stop=op["stop"]).then_inc(s_pe, 1)

        @block.scalar
        def _(a):
            wait = make_waiter(a)
            dmo_cnt = {}
            for op in pl.actq:
                wait(op["waits"])
                if op["kind"] == "dmo":
                    k = op["key"]
                    dmo_cnt[k] = dmo_cnt.get(k, 0) + 16
                    a.dma_start(out=ap(op["dst"], op["dst_sl"]),
                                in_=ap(op["src"], op["src_sl"])).then_inc(dma_sems[k], 16)
                elif op["bias"] is None:
                    a.activation(ap(op["out"], op["out_sl"]), ap(op["in"], op["in_sl"]),
                                 op["func"], scale=op["scale"]).then_inc(s_act, 1)
                else:
                    bi, bc = op["bias"]
                    bias_ap = bias_b[bi][:, bc:bc + 1].bitcast(F32)
                    a.activation(ap(op["out"], op["out_sl"]), ap(op["in"], op["in_sl"]),
                                 op["func"], bias=bias_ap,
                                 scale=op["scale"]).then_inc(s_act, 1)
            for k, v in sorted(dmo_cnt.items()):
                a.wait_ge(dma_sems[k], v)

        @block.vector
        def _(v):
            wait = make_waiter(v)
            for op in pl.dve:
                wait(op["waits"])
                if op.get("kind") == "mul":
                    v.tensor_mul(ap(op["out"], op["out_sl"]),
                                 ap(op["in"], op["in_sl"]),
                                 ap(op["in2"], op["in2_sl"])).then_inc(s_dve, 1)
                else:
                    v.tensor_copy(ap(op["out"], op["out_sl"]),
                                  ap(op["in"], op["in_sl"])).then_inc(s_dve, 1)

    return nc


# ---------------------------------------------------------------------------
# Host-side routing, gathering, execution, unsharding
# ---------------------------------------------------------------------------
def plan_all(cat_ids):
    """Group items by category, split into (cat, items, half) units, balance
    over cores. Returns (units, per_core, nslot, ips)."""
    order = {}
    for b, g in enumerate(cat_ids.tolist()):
        order.setdefault(g, []).append(b)
    ips = max(1, min(4, max(len(v) for v in order.values())))
    units = []
    for g in sorted(order):
        items = order[g]
        for i0 in range(0, len(items), ips):
            grp = items[i0:i0 + ips]
            for h in range(2):
                units.append((g, grp, h))
    nslot = max(1, -(-len(units) // N_CORES))
    per_core = [[] for _ in range(N_CORES)]
    for i, u in enumerate(units):
        per_core[i % N_CORES].append(u)
    for c in range(N_CORES):
        while len(per_core[c]) < nslot:
            per_core[c].append(None)  # dummy
    return units, per_core, nslot, ips


def make_inputs(units_core, nslot, ips, state, actions, tau_np,
                se_W1, se_b1, se_W2, se_b2,
                ae_W1, ae_b1, ae_W2, ae_b2, ae_W3, ae_b3, with_bias=False):
    TOK = ips * T
    OFF = _blob_offsets(ips)
    PIN_W = OFF["PIN_W"]
    PIN_TAU, PIN_ACT = 0, 48
    PIN_ST = PIN_ACT + TOK
    z = np.zeros
    f = np.float32
    cst_f = z((128, 256), f)
    cst_f[0:ips, 0:TOK] = np.kron(np.eye(ips, dtype=f), np.ones((1, T), f))
    cst_f[0, 128:] = 1.0
    spb = z((nslot, 128, OFF["SPW"]), np.uint8)
    acb = z((nslot, 128, OFF["ACW"]), np.uint8)
    d = {"spblob": spb, "acblob": acb, "cst_f": cst_f}

    def put(blob, s, byte_off, arr):
        raw = arr.view(np.uint8)
        blob[s, :raw.shape[0], byte_off:byte_off + raw.shape[1]] = raw

    def chunk_major(w, groups, chunks, width):
        # [groups*chunks*128, width] -> [groups, 128, chunks*width]
        return (w.reshape(groups, chunks, 128, width)
                .transpose(0, 2, 1, 3).reshape(groups, 128, chunks * width))
    if with_bias:
        d["biasd"] = z((nslot, 128, 3872), f)
    for s, (g, items, h) in enumerate(units_core):
        H = slice(h * HH, (h + 1) * HH)
        O = slice(h * OH, (h + 1) * OH)
        # wsea [96, 1024]: se_W1 half at rows 0:64 cols 0:512; ae_W1 chunk j
        # at rows 32*(j//4) : +32, cols 512+(j%4)*128 (bases 0/32/64 only)
        wsea = np.zeros((96, 1024), f)
        wsea[:STATE_DIM, 0:HH] = se_W1[g][:, H]
        for j in range(12):
            r0, c0 = 32 * (j // 4), 512 + (j % 4) * 128
            wsea[r0:r0 + ACT_DIM, c0:c0 + 128] = ae_W1[g][:, j * 128:(j + 1) * 128]
        put(spb, s, OFF["W"], _bf16(wsea))
        se2 = chunk_major(se_W2[g][H, :], 2, 2, EMB)
        w2bg = chunk_major(ae_W2[g][EMB:, O], 2, 6, OH)
        w2ag = chunk_major(ae_W2[g][:EMB, O], 2, 6, OH)
        w3g = chunk_major(ae_W3[g][O, :], 2, 3, EMB) * (1.0 / S2)
        qse2 = _fp8(se2, S_W) if FP8_WSE2 else _bf16(se2)
        q2b = _fp8(w2bg, S_W) if FP8_W2B else _bf16(w2bg)
        q2a = _fp8(w2ag, S_W) if FP8_W2A else _bf16(w2ag)
        put(spb, s, OFF["S2"], qse2[0])
        put(spb, s, OFF["2B"], q2b[0])
        put(spb, s, OFF["2A"], q2a[0])
        put(spb, s, OFF["3"], _bf16(w3g[0]))
        put(acb, s, OFF["aS2"], qse2[1])
        put(acb, s, OFF["a2B"], q2b[1])
        put(acb, s, OFF["a2A"], q2a[1])
        put(acb, s, OFF["a3"], _bf16(w3g[1]))
        p = np.zeros((128, PIN_W), f)
        tau3 = p[:, PIN_TAU:PIN_TAU + 12 * ips].reshape(128, 12, ips)
        for i, b in enumerate(items):
            tau3[:, :, i] = (tau_np[b] * TAU_SCALE).reshape(12, 128).T
            # actionsT replicated at the three 32-row base partitions the
            # wsea AE1 chunks live at (lhsT/rhs bases must match)
            for rr in range(3):
                p[rr * 32:(rr + 1) * 32, PIN_ACT + i * T:PIN_ACT + (i + 1) * T] = actions[b].T
            p[0:STATE_DIM, PIN_ST + i] = state[b, 0]
        put(spb, s, OFF["P"], _bf16(p))
        if with_bias:
            bb = d["biasd"][s]
            for j in range(12):
                bb[:, j] = S_AT * ae_b1[g][j * 128:(j + 1) * 128]
            for k in range(4):
                bb[:, 12 + k] = S_H * se_b1[g][H][k * 128:(k + 1) * 128]
            bb[0, 16:16 + OH] = S2 * ae_b2[g][O]
            if h == 0:
                bb[0, 800:800 + EMB] = ae_b3[g]
                bb[0, 2336:2336 + EMB] = S1 * se_b2[g]
    return d


def kernel(state, actions, timesteps, cat_ids,
           se_W1, se_b1, se_W2, se_b2,
           ae_W1, ae_b1, ae_W2, ae_b2, ae_W3, ae_b3):
    args = [np.asarray(a) for a in (state, actions, timesteps, cat_ids, se_W1, se_b1,
                                    se_W2, se_b2, ae_W1, ae_b1, ae_W2, ae_b2, ae_W3, ae_b3)]
    (state, actions, timesteps, cat_ids, se_W1, se_b1, se_W2, se_b2,
     ae_W1, ae_b1, ae_W2, ae_b2, ae_W3, ae_b3) = args
    tau_np = _sinusoid(timesteps)

    units, per_core, nslot, ips = plan_all(cat_ids)
    with_bias = bool(any(np.any(a) for a in (se_b1, ae_b1, ae_b2, ae_b3, se_b2)))
    in_maps = []
    for c in range(N_CORES):
        units_c = [(u if u is not None else units[0]) for u in per_core[c]]
        in_maps.append(make_inputs(units_c, nslot, ips, state, actions, tau_np,
                                   se_W1, se_b1, se_W2, se_b2,
                                   ae_W1, ae_b1, ae_W2, ae_b2, ae_W3, ae_b3,
                                   with_bias=with_bias))

    nc = build(nslot, ips=ips, with_bias=with_bias)
    res = run_bass_kernel_spmd(nc, in_maps, list(range(N_CORES)))

    TOK = ips * T
    merged = (TOK + ips) <= 128
    out = np.zeros((B, T + 1, EMB), np.float32)
    st_scale = 1.0 / S1
    for c in range(N_CORES):
        oo = res.results[c]["o"].astype(np.float32)
        stx = None if merged else res.results[c]["st"].astype(np.float32)
        for s, u in enumerate(per_core[c]):
            if u is None:
                continue
            g, items, h = u
            for i, b in enumerate(items):
                sf = oo[s, TOK + i] if merged else stx[s, i]
                out[b, 0] += sf * st_scale
                out[b, 1:] += oo[s, i * T:(i + 1) * T]
    return out
